# revision 12
# baseline (speedup 1.0000x reference)
"""Trainium2 Bass kernel for nn_CrossAttentionDown (region-RoPE cross attention).

Full-input contract: kernel(**inputs) takes the complete tensors, shards
(B, H) across 8 NeuronCores (each core: one batch, half the heads), runs an
SPMD Bass kernel, and gathers the full [B, H, P, D] output.

Math notes (vs the jax reference):
 - softmax(x + c) == softmax(x) per row, so the per-head bias_diff constant
   drops out; only delta_h = bias_same - bias_diff matters (computed on the
   host). It is folded into the QK^T contraction via 32 extra dims: K side
   gets onehot(regions[t]==n) (written once per core), Q side gets
   delta_h * onehot(n == p//4).
 - scores are computed transposed ([t, p] layout) so both the exp output and
   V can feed the AV matmul with t on the contraction (partition) dim. The
   AV matmul uses exp(scores) tiles as the stationary operand, so the output
   lands directly as [p, d]; the softmax denominator is one extra
   ones-column matmul sharing the same stationary tile.
 - tokens are tiled contiguously across partitions (token = 32*p + jj) so
   K/V DMA reads 8KB contiguous per partition; attention is invariant to
   the token permutation as long as K, V, regions and the rope/bias tables
   use the same ordering.
 - rope on K: the global-position half uses compile-time cos/sin tables
   (positions are static; DMA'd compact and pair-expanded on DVE). The
   region half has only 32 distinct angles (region id 1..32 x 16 freqs), so
   its cos/sin are a compile-time [32, 32] table gathered per token with
   32 small PE matmuls against the one-hot rows (no on-device range
   reduction / Sin for K at all). The rotation itself is
   out = k*chat + pairswap(k)*shat with all-fp16 packed operands (DVE 2x),
   combined via scalar_tensor_tensor (DVE 4x); pairswap is a
   negative-stride access-pattern view.
 - region starts (first t with regions==n, 0 if absent) are computed on
   device: per-(t%4, region) counts via 8 accumulating [128,128] matmuls
   with a ones vector, then an exclusive prefix sum via constant
   4x-replicated triangular matmuls (regions are sorted).
 - Q's region-half rope angles (ridx = p//4+1) are compile-time; only the
   gpos half needs the on-device range-reduce + Sin chain ([128, 16]).
"""

import sys

if "/opt/trn_rl_repo" not in sys.path:
    sys.path.insert(0, "/opt/trn_rl_repo")

import math

import numpy as np

B, H, T, D = 4, 16, 4096, 64
MAX_N = 32
R_TOK = 4
P = MAX_N * R_TOK  # 128 pool queries
NCORES = 8
HPC = H // 2  # heads per core
NT = T // 128  # 32 token tiles of 128
TPP = T // 128  # tokens per partition = 32
NPAIR = 16  # rotation pairs per half (half dim = 32)
KAUG = D + MAX_N  # 96 contraction dims (64 rot + 32 bias one-hot)
THETA = 10000.0

_cache = {}


def _split_waits(nc, maxw=1):
    """The pinned walrus rejects instructions with more than one embedded
    semaphore wait. Hoist excess waits into preceding wait-only Drain
    instructions on the same engine (same-engine program order preserves
    the blocking semantics)."""
    import concourse.mybir as mybir

    n_new = 0
    for f in nc.m.functions:
        for blk in f.blocks:
            new_list = []
            for inst in blk.instructions:
                si = getattr(inst, "sync_info", None)
                waits = list(si.on_wait) if si is not None and si.on_wait else []
                if len(waits) > maxw:
                    excess, keep = waits[:-maxw], waits[-maxw:]
                    for j, w in enumerate(excess):
                        d = mybir.InstDrain(name=f"{inst.name}-w{j}", ins=[], outs=[])
                        d.engine = inst.engine
                        d.sync_info = mybir.SyncInfo(on_wait=[w], on_update=[])
                        d.debug = inst.debug
                        new_list.append(d)
                        n_new += 1
                    si.on_wait = keep
                new_list.append(inst)
            blk.instructions[:] = new_list
    return n_new


def _emit_range_reduce(nc, mybir, pool, ang, ncols, name):
    """In-place reduce ang (>=0, < ~2^20) to [-pi, pi] mod 2pi. k is computed
    with the fp32 magic-number trick (guaranteed round-to-nearest), then a
    two-term Cody-Waite subtraction (hi=6.28125, k*hi exact for small k),
    then a clamp for boundary epsilon."""
    f32 = mybir.dt.float32
    INV2PI = float(np.float32(1.0 / (2.0 * math.pi)))
    HI = 6.28125
    LO = float(np.float32(2.0 * math.pi - HI))
    PI = float(np.float32(math.pi))
    MAGIC = float(np.float32(1.5 * 2.0**23))
    kf = pool.tile([128, ncols], f32, name=f"{name}_kf", tag=f"{name}_kf")
    nc.vector.tensor_scalar(
        kf[:], ang, INV2PI, MAGIC, op0=mybir.AluOpType.mult, op1=mybir.AluOpType.add
    )
    nc.vector.tensor_scalar_add(kf[:], kf[:], -MAGIC)
    nc.vector.scalar_tensor_tensor(
        ang, kf[:], -HI, ang, op0=mybir.AluOpType.mult, op1=mybir.AluOpType.add
    )
    nc.vector.scalar_tensor_tensor(
        ang, kf[:], -LO, ang, op0=mybir.AluOpType.mult, op1=mybir.AluOpType.add
    )
    mt = pool.tile([128, ncols], f32, name=f"{name}_mt", tag=f"{name}_mt")
    nc.vector.tensor_scalar(mt[:], ang, PI, None, op0=mybir.AluOpType.is_gt)
    nc.vector.scalar_tensor_tensor(
        ang, mt[:], -(HI + LO), ang,
        op0=mybir.AluOpType.mult, op1=mybir.AluOpType.add,
    )
    nc.vector.tensor_scalar(mt[:], ang, -PI, None, op0=mybir.AluOpType.is_lt)
    nc.vector.scalar_tensor_tensor(
        ang, mt[:], (HI + LO), ang,
        op0=mybir.AluOpType.mult, op1=mybir.AluOpType.add,
    )


def _emit_sincos(nc, mybir, pool, presb_tile_fn, ang, ncols, name, AF):
    """Given ang in [-pi, pi], produce (sin, cos) tiles; cos = sin of the
    +pi/2-shifted, re-wrapped angle (clobbers ang)."""
    import math as _math

    f32 = mybir.dt.float32
    sin_t = presb_tile_fn([128, ncols], f32, f"{name}_sin")
    nc.scalar.activation(sin_t[:], ang, AF.Sin)
    nc.vector.tensor_scalar_add(ang, ang, float(_math.pi / 2))
    mt = presb_tile_fn([128, ncols], f32, f"{name}_mt2")
    nc.vector.tensor_scalar(
        mt[:], ang, float(np.float32(_math.pi)), None, op0=mybir.AluOpType.is_gt
    )
    nc.vector.scalar_tensor_tensor(
        ang, mt[:], float(-2.0 * _math.pi), ang,
        op0=mybir.AluOpType.mult, op1=mybir.AluOpType.add,
    )
    cos_t = presb_tile_fn([128, ncols], f32, f"{name}_cos")
    nc.scalar.activation(cos_t[:], ang, AF.Sin)
    return sin_t, cos_t


def _build_program():
    import concourse.bass as bass
    import concourse.mybir as mybir
    import concourse.tile as tile

    f32 = mybir.dt.float32
    f16 = mybir.dt.float16  # 16-bit matmul dtype (fp16: 11-bit mantissa)
    AF = mybir.ActivationFunctionType
    ALU = mybir.AluOpType

    nc = bass.Bass("TRN2", target_bir_lowering=False, debug=False)

    q_d = nc.dram_tensor("q", [HPC, P, D], f32, kind="ExternalInput")
    k_d = nc.dram_tensor("k", [HPC, T, D], f32, kind="ExternalInput")
    v_d = nc.dram_tensor("v", [HPC, T, D], f32, kind="ExternalInput")
    reg_d = nc.dram_tensor("regions_f", [T], f32, kind="ExternalInput")
    del_d = nc.dram_tensor("delta8", [HPC], f16, kind="ExternalInput")
    out_d = nc.dram_tensor("out", [HPC, P, D], f32, kind="ExternalOutput")

    # ---- compile-time constants ----
    inv = (1.0 / (THETA ** (np.arange(0, 32, 2, dtype=np.float64) / 32.0))).astype(
        np.float64
    )  # [16] rope inverse freqs (per half, half dim 32)
    # token(p, jj) = 32*p + jj ; position-half cos/sin, expanded to pair slots
    tok = (32.0 * np.arange(128, dtype=np.float64)[:, None]
           + np.arange(TPP, dtype=np.float64)[None, :])  # [128, 32]
    ang1 = tok[:, :, None] * inv[None, None, :]  # [128,32,16]
    c1_np = np.repeat(np.cos(ang1), 2, axis=-1).reshape(128, TPP * 32)
    s1_half = np.sin(ang1)
    s1_np = np.stack([-s1_half, s1_half], axis=-1).reshape(128, TPP * 32)
    c1s1_np = np.concatenate([c1_np, s1_np], axis=1).astype(np.float16)  # [128, 2048]

    # region-half table: region ids 1..32 (exactly 32 distinct angles per freq)
    nvals = np.arange(1, MAX_N + 1, dtype=np.float64)  # [32]
    angr = nvals[:, None] * inv[None, :]  # [32, 16]
    # rows 64:96 so the gather matmul's operands share a base partition
    # with kta[64:96] (the one-hot rows)
    tblcs_np = np.zeros((128, 32), np.float16)
    tblcs_np[D : D + MAX_N, 0:16] = np.cos(angr).astype(np.float16)
    tblcs_np[D : D + MAX_N, 16:32] = np.sin(angr).astype(np.float16)

    ident_np = np.eye(128, dtype=np.float16)
    ones16_np = np.ones((128, 1), np.float16)
    # 4x-replicated prefix/select matrices: rows (tt*32 + n), cols p
    atpre_np = (np.arange(MAX_N)[:, None] < (np.arange(128)[None, :] // R_TOK)
                ).astype(np.float16)  # [32, 128]
    onehotP_np = (np.arange(MAX_N)[:, None] == (np.arange(128)[None, :] // R_TOK)
                  ).astype(np.float16)  # [32, 128]
    atpre4_np = np.tile(atpre_np, (4, 1))  # [128, 128]
    onehotP4_np = np.tile(onehotP_np, (4, 1))  # [128, 128]

    # blkS (f16): ident | ones | tbl | atpre4 | onehotP4   [128, 417]
    blkS_np = np.concatenate(
        [ident_np, ones16_np, tblcs_np, atpre4_np, onehotP4_np], axis=1
    )

    # blkA (f32): inv128 | nids128 | cqr | sqr   [128, 112]
    inv128_np = np.broadcast_to(inv.astype(np.float32)[None, :], (128, NPAIR))
    nids128_np = np.broadcast_to(
        np.arange(1, MAX_N + 1, dtype=np.float32)[None, :], (128, MAX_N)
    )
    # Q region half: ridx = p//4 + 1 (compile-time), with the 1/8 score scale
    ridx = (np.arange(128, dtype=np.float64) // R_TOK + 1.0)  # [128]
    angqr = ridx[:, None] * inv[None, :]  # [128, 16]
    cqr_np = 0.125 * np.repeat(np.cos(angqr), 2, axis=-1)  # [128, 32]
    sqr_half = 0.125 * np.sin(angqr)
    sqr_np = np.stack([-sqr_half, sqr_half], axis=-1).reshape(128, 32)
    blkA_np = np.concatenate(
        [inv128_np, nids128_np, cqr_np, sqr_np], axis=1
    ).astype(np.float32)  # [128, 112]

    blkS_c = nc.inline_tensor(blkS_np, name="blkS_c")
    blkA_c = nc.inline_tensor(blkA_np, name="blkA_c")
    c1s1_c = nc.inline_tensor(c1s1_np, name="c1s1_c")

    with tile.TileContext(nc) as tc:
        with tc.tile_pool(name="const", bufs=1) as cpool:
            blkS = cpool.tile([128, 417], f16, name="blkS")
            blkA = cpool.tile([128, 112], f32, name="blkA")
            c1s1 = cpool.tile([128, 2 * TPP * 32], f16, name="c1s1")
            cfs = cpool.tile([128, 2 * TPP * D], f16, name="cfs")
            ident = blkS[:, 0:128]
            onesf16 = blkS[:, 128:129]
            tblcs = blkS[D : D + MAX_N, 129:161]
            atpre4 = blkS[:, 161:289]
            onehotP4 = blkS[:, 289:417]
            onehotP = blkS[0:MAX_N, 289:417]
            inv128 = blkA[:, 0:NPAIR]
            nids128 = blkA[:, NPAIR : NPAIR + MAX_N]
            cqr = blkA[:, 48:80]
            sqr = blkA[:, 80:112]
            cfull = cfs[:, 0 : TPP * D]
            sfull = cfs[:, TPP * D : 2 * TPP * D]

            with tc.tile_pool(name="tables", bufs=1) as tpool:
                # persistent per-core tables
                kta = tpool.tile([KAUG, 2 * T], f16, name="kta")  # dbl-buffered by head parity
                qT_all = tpool.tile([KAUG, HPC * 128], f16, name="qT_all")
                gpos = tpool.tile([128, 1], f32, name="gpos")
                delta32 = tpool.tile([MAX_N, HPC], f16, name="delta32")

                # DMA issue order (HWDGE generates descriptors serially at
                # ~625ns/DMA): head-0 K first, then the preamble inputs
                # (regions + consts feed the whole table build), then v0/q,
                # then the remaining K/V stream. Out-DMAs go on the ACT
                # queue so they cannot head-of-line-block these.
                prio = tc.alloc_tile_pool(name="pre_io", bufs=1)
                iopool = tc.alloc_tile_pool(name="io", bufs=3)
                ksbs, vsbs = [], []
                for h in range(HPC):
                    ksbs.append(iopool.tile([128, TPP * D], f32, name="ksb", tag="ksb"))
                    vsbs.append(iopool.tile([128, TPP * D], f32, name="vsb", tag="vsb"))

                regf = prio.tile([128, TPP], f32, name="regf")
                nc.sync.dma_start(regf[:], reg_d.ap().rearrange("(p t) -> p t", t=TPP))
                nc.sync.dma_start(blkS[:], blkS_c.ap())
                nc.sync.dma_start(blkA[:], blkA_c.ap())
                nc.sync.dma_start(
                    ksbs[0].rearrange("p (t d) -> p t d", t=TPP),
                    k_d.ap()[0].rearrange("(p t) d -> p t d", t=TPP),
                )
                nc.sync.dma_start(c1s1[:], c1s1_c.ap())
                nc.sync.dma_start(
                    vsbs[0].rearrange("p (t d) -> p t d", t=TPP),
                    v_d.ap()[0].rearrange("(p t) d -> p t d", t=TPP),
                )
                qsb = prio.tile([128, HPC * D], f32, name="qsb")
                nc.sync.dma_start(
                    qsb.rearrange("p (h d) -> p h d", h=HPC),
                    q_d.ap().rearrange("h p d -> p h d"),
                )
                nc.sync.dma_start(
                    delta32[:],
                    del_d.ap().rearrange("(o h) -> o h", o=1).broadcast_to([MAX_N, HPC]),
                )
                for h in range(1, HPC):
                    nc.sync.dma_start(
                        ksbs[h].rearrange("p (t d) -> p t d", t=TPP),
                        k_d.ap()[h].rearrange("(p t) d -> p t d", t=TPP),
                    )
                    nc.sync.dma_start(
                        vsbs[h].rearrange("p (t d) -> p t d", t=TPP),
                        v_d.ap()[h].rearrange("(p t) d -> p t d", t=TPP),
                    )

                # main-loop pools allocated BEFORE the preamble scratch so
                # the pipeline's SBUF/PSUM does not alias preamble tiles
                # (aliasing would serialize the first heads behind preamble
                # readers). Preamble matmuls borrow pipeline PSUM tiles.
                wpool = tc.alloc_tile_pool(name="work", bufs=3)
                vpool = tc.alloc_tile_pool(name="vwork", bufs=4)
                apool = tc.alloc_tile_pool(name="attn", bufs=4)
                fpool = tc.alloc_tile_pool(name="fin", bufs=2)
                ktps = tc.alloc_tile_pool(name="kt_ps", bufs=2, space="PSUM")
                scps = tc.alloc_tile_pool(name="sc_ps", bufs=2, space="PSUM")
                avps = tc.alloc_tile_pool(name="av_ps", bufs=2, space="PSUM")

                # pre-allocate the 4 rotating V buffers and write their ones
                # columns once here (Pool is idle in the preamble); the
                # per-head cast never touches the ones column, so this keeps
                # the 122ns memset off the Pool steady-state cadence.
                vbfs = []
                for _vb in range(4):
                    vbf = vpool.tile([128, TPP * (D + 1)], f16, name="vbf", tag="vbf")
                    nc.gpsimd.memset(
                        vbf.rearrange("p (t d) -> p t d", t=TPP)[:, :, D : D + 1], 1.0
                    )
                    vbfs.append(vbf)

                with tc.tile_pool(name="pre_sb", bufs=1) as presb:
                    # ---- one-hot of region ids, in 4 column chunks so the
                    # transposes can start before the whole tile is done.
                    # oh[q, jj, n] = (regions[32q+jj] == n+1), chunk = 8 jj.
                    ohcs = []
                    with tc.high_priority():
                        for c in range(4):
                            ohc = presb.tile([128, 8 * MAX_N], f16, name=f"ohc{c}")
                            nc.vector.tensor_tensor(
                                ohc.rearrange("p (t n) -> p t n", n=MAX_N),
                                regf[:, 8 * c : 8 * (c + 1), None].broadcast_to(
                                    [128, 8, MAX_N]
                                ),
                                nids128[:, None, :].broadcast_to([128, 8, MAX_N]),
                                op=ALU.is_equal,
                            )
                            ohcs.append(ohc)

                    # ---- region-start counts: 8 accumulating [128,128]
                    # matmuls vs a ones vector -> cnt128[(t%4)*32+n, 1]
                    cnt_ps = avps.tile([128, D + 1], f32, name="avp", tag="avp")
                    for c in range(4):
                        for half in range(2):
                            nc.tensor.matmul(
                                cnt_ps[:, 0:1],
                                ohcs[c][:, half * 128 : (half + 1) * 128],
                                onesf16,
                                start=(c == 0 and half == 0),
                                stop=(c == 3 and half == 1),
                            )
                    cnt_sb = presb.tile([128, 1], f16, name="cnt_sb")
                    nc.vector.tensor_copy(cnt_sb[:], cnt_ps[:, 0:1])

                    # starts (exclusive prefix over counts, 4x-replicated
                    # triangular matrices fold the t%4 split) -> gpos
                    st_ps = avps.tile([128, D + 1], f32, name="avp", tag="avp")
                    nc.tensor.matmul(st_ps[:, 0:1], atpre4, cnt_sb[:], start=True, stop=True)
                    nc.tensor.matmul(st_ps[:, 4:5], onehotP4, cnt_sb[:], start=True, stop=True)
                    gtm = presb.tile([128, 1], f32, name="gtm")
                    nc.vector.tensor_scalar(
                        gtm[:], st_ps[:, 4:5], 0.0, None, op0=ALU.is_gt
                    )
                    nc.vector.tensor_mul(gpos[:], st_ps[:, 0:1], gtm[:])

                    # ---- position-half K tables: expand compact c1/s1 into
                    # the pair-slot layout (region slots filled later by the
                    # gather path). DVE 4x copies.
                    cf_v = cfull.rearrange("p (t c) -> p t c", t=TPP)
                    sf_v = sfull.rearrange("p (t c) -> p t c", t=TPP)
                    nc.vector.tensor_copy(
                        cf_v[:, :, 0:32],
                        c1s1[:, 0 : TPP * 32].rearrange("p (t c) -> p t c", t=TPP),
                    )
                    nc.vector.tensor_copy(
                        sf_v[:, :, 0:32],
                        c1s1[:, TPP * 32 : TPP * 64].rearrange("p (t c) -> p t c", t=TPP),
                    )

                    # ---- one-hot transposes -> kta rows 64:96 (buf0), then
                    # gather matmuls against the compile-time cos/sin table,
                    # then pair-slot expansion into cfs region slots. This is
                    # the critical path to head 0's rotate.
                    csps = scps.tile([128, 1024], f32, name="scp", tag="scp")
                    with tc.high_priority():
                        kta_oh = kta[D:KAUG, 0:T]
                        for g in range(2):
                            ohp = ktps.tile([128, 1024], f16, name="ktp", tag="ktp")
                            for i in range(8):
                                t2 = g * 8 + i  # covers jj = 2*t2, 2*t2+1
                                c = t2 // 4
                                lo = (t2 % 4) * 64
                                nc.tensor.transpose(
                                    ohp[0:64, i * 128 : (i + 1) * 128],
                                    ohcs[c][:, lo : lo + 64],
                                    ident,
                                )
                            kta_g = kta_oh[:, g * 2048 : (g + 1) * 2048]
                            kta_v = kta_g.rearrange("n (i e o) -> n i e o", i=8, e=2)
                            ohp_e = ohp[0:MAX_N, :].rearrange("n (i o) -> n i o", i=8)
                            ohp_o = ohp[MAX_N : 2 * MAX_N, :].rearrange(
                                "n (i o) -> n i o", i=8
                            )
                            if g == 0:
                                nc.vector.tensor_copy(kta_v[:, :, 0, :], ohp_e)
                                nc.scalar.activation(kta_v[:, :, 1, :], ohp_o, AF.Copy)
                            else:
                                nc.scalar.activation(kta_v[:, :, 0, :], ohp_e, AF.Copy)
                                nc.vector.tensor_copy(kta_v[:, :, 1, :], ohp_o)
                            for jj in range(g * 16, (g + 1) * 16):
                                nc.tensor.matmul(
                                    csps[:, jj * 32 : (jj + 1) * 32],
                                    kta_oh[:, jj * 128 : (jj + 1) * 128],
                                    tblcs,
                                    start=True,
                                    stop=True,
                                )
                        # expansion: cfs region slots <- gathered cos/sin
                        # (split across DVE and ACT to shorten the serial
                        # chain in front of head 0's rotate)
                        cs_v = csps.rearrange("p (t c) -> p t c", t=TPP)
                        c2_v = cf_v[:, :, 32:64].rearrange("p t (j e) -> p t j e", e=2)
                        s2_v = sf_v[:, :, 32:64].rearrange("p t (j e) -> p t j e", e=2)
                        nc.vector.tensor_copy(s2_v[:, :, :, 1], cs_v[:, :, 16:32])
                        nc.scalar.activation(
                            c2_v,
                            cs_v[:, :, 0:16, None].broadcast_to([128, TPP, NPAIR, 2]),
                            AF.Copy,
                        )
                        nc.scalar.mul(s2_v[:, :, :, 0], cs_v[:, :, 16:32], -1.0)

                    # ---- Q-side: gpos-half angles (data-dependent), region
                    # half from compile-time cqr/sqr
                    angq = presb.tile([128, NPAIR], f32, name="angq")
                    nc.vector.tensor_scalar_mul(angq[:], inv128, gpos[:])
                    _emit_range_reduce(nc, mybir, presb, angq[:], NPAIR, "rrq")
                    sinq, cosq = _emit_sincos(
                        nc, mybir, presb,
                        lambda s, d, n: presb.tile(s, d, name=n),
                        angq[:], NPAIR, "rrq", AF,
                    )
                    cq = presb.tile([128, D], f32, name="cq")
                    sq = presb.tile([128, D], f32, name="sq")
                    cq_v = cq[:, 0:32].rearrange("p (j e) -> p j e", e=2)
                    sq_v = sq[:, 0:32].rearrange("p (j e) -> p j e", e=2)
                    nc.scalar.mul(
                        cq_v, cosq[:, :, None].broadcast_to([128, NPAIR, 2]), 0.125
                    )
                    nc.scalar.mul(sq_v[:, :, 1], sinq[:], 0.125)
                    nc.scalar.mul(sq_v[:, :, 0], sinq[:], -0.125)
                    nc.scalar.activation(cq[:, 32:64], cqr, AF.Copy)
                    nc.scalar.activation(sq[:, 32:64], sqr, AF.Copy)

                    # rotate all q heads: qrot = q*cq + swap(q)*sq
                    qs_v = qsb.rearrange("p (h d) -> p h d", h=HPC)
                    qs_swap = qsb.rearrange("p (h j e) -> p h j e", h=HPC, e=2)[
                        :, :, :, ::-1
                    ]
                    qrot = presb.tile([128, HPC * D], f16, name="qrot")
                    qtm = presb.tile([128, HPC * D], f16, name="qtm")
                    qr_v = qrot.rearrange("p (h d) -> p h d", h=HPC)
                    sq_v4 = sq.rearrange("p (j e) -> p j e", e=2)[:, None, :, :]
                    nc.vector.tensor_mul(
                        qr_v, qs_v, cq[:, None, :].broadcast_to([128, HPC, D])
                    )
                    nc.vector.tensor_mul(
                        qtm.rearrange("p (h j e) -> p h j e", h=HPC, e=2),
                        qs_swap,
                        sq_v4.broadcast_to([128, HPC, 2 * NPAIR, 2]),
                    )
                    nc.vector.tensor_add(qrot[:], qrot[:], qtm[:])

                    # transpose q (8 heads) into one psum bank, copy once
                    qtp = ktps.tile([128, 1024], f16, name="ktp", tag="ktp")
                    qr_h = qrot.rearrange("p (h d) -> p h d", h=HPC)
                    for h in range(HPC):
                        nc.tensor.transpose(
                            qtp[0:D, h * 128 : (h + 1) * 128], qr_h[:, h, :], ident
                        )
                    nc.vector.tensor_copy(qT_all[0:D, :], qtp[0:D, :])

                    # bias rows: qT[64+n, h*128+p] = delta[n, h] * onehotP[n, p]
                    qb_v = qT_all[D:KAUG, :].rearrange("n (h p) -> n h p", h=HPC)
                    nc.vector.tensor_mul(
                        qb_v,
                        onehotP[:, None, :].broadcast_to([MAX_N, HPC, 128]),
                        delta32[:, :, None].broadcast_to([MAX_N, HPC, 128]),
                    )

                # ================= main per-head loop =================
                for h in range(HPC):
                    kb = (h % 2) * T  # kta column base for this head
                    ksb, vsb = ksbs[h], vsbs[h]

                    # f32 -> fp16 casts on gpsimd (otherwise idle);
                    # vbf keeps a ones column per tile for the softmax
                    # denominator
                    kbf = wpool.tile([128, TPP * D], f16, name="kbf", tag="kbf")
                    nc.gpsimd.tensor_copy(kbf[:], ksb[:])
                    vbf = vbfs[h % 4]
                    vb_t = vbf.rearrange("p (t d) -> p t d", t=TPP)
                    nc.gpsimd.tensor_copy(
                        vb_t[:, :, 0:D], vsb.rearrange("p (t d) -> p t d", t=TPP)
                    )
                    if h == 1:
                        # one-hot rows for the parity-1 buffer (DVE 4x copy)
                        nc.vector.tensor_copy(
                            kta[D:KAUG, T : 2 * T], kta[D:KAUG, 0:T]
                        )

                    # rotate K: kra = kbf*c + pairswap(kbf)*s (fp16 packed
                    # muls at DVE 2x, combine via scalar_tensor_tensor 4x)
                    kra = wpool.tile([128, TPP * D], f16, name="kra", tag="kra")
                    tmp = wpool.tile([128, TPP * D], f16, name="tmp", tag="tmp")
                    ksw = kbf.rearrange("p (t j e) -> p t j e", t=TPP, e=2)[
                        :, :, :, ::-1
                    ]
                    nc.vector.tensor_mul(kra[:], kbf[:], cfull)
                    nc.vector.tensor_mul(
                        tmp.rearrange("p (t j e) -> p t j e", t=TPP, e=2),
                        ksw,
                        sfull.rearrange("p (t j e) -> p t j e", t=TPP, e=2),
                    )
                    nc.vector.tensor_add(kra[:], kra[:], tmp[:])

                    # transpose: 2 tiles per [128,128] PE transpose, 8 pairs
                    # per psum group; unpack even/odd tiles with strided
                    # copies (one of the four on ACT to keep DVE under the
                    # DMA cadence)
                    for g in range(2):
                        ktp = ktps.tile([128, 1024], f16, name="ktp", tag="ktp")
                        for i in range(8):
                            t2 = g * 8 + i  # covers k-tiles 2*t2, 2*t2+1
                            nc.tensor.transpose(
                                ktp[:, i * 128 : (i + 1) * 128],
                                kra[:, (2 * t2) * D : (2 * t2 + 2) * D],
                                ident,
                            )
                        kta_g = kta[0:D, kb + g * 2048 : kb + (g + 1) * 2048]
                        kta_v = kta_g.rearrange("c (i e o) -> c i e o", i=8, e=2)
                        ktp_e = ktp[0:D, :].rearrange("c (i o) -> c i o", i=8)
                        ktp_o = ktp[D:128, :].rearrange("c (i o) -> c i o", i=8)
                        nc.vector.tensor_copy(kta_v[:, :, 0, :], ktp_e)
                        if g == 0:
                            nc.vector.tensor_copy(kta_v[:, :, 1, :], ktp_o)
                        else:
                            nc.scalar.activation(kta_v[:, :, 1, :], ktp_o, AF.Copy)

                    # scores (transposed), exp, AV accumulation
                    at = apool.tile([128, T], f16, name="at", tag="at")
                    avp = avps.tile([128, D + 1], f32, name="avp", tag="avp")
                    for g in range(4):
                        scp = scps.tile([128, 1024], f32, name="scp", tag="scp")
                        for i in range(8):
                            t = g * 8 + i
                            nc.tensor.matmul(
                                scp[:, i * 128 : (i + 1) * 128],
                                kta[0:KAUG, kb + t * 128 : kb + (t + 1) * 128],
                                qT_all[0:KAUG, h * 128 : (h + 1) * 128],
                                start=True,
                                stop=True,
                            )
                        nc.scalar.activation(
                            at[:, g * 1024 : (g + 1) * 1024], scp[:], AF.Exp
                        )
                    # AV after all score groups: exp(g) overlaps scores(g+1)
                    # instead of stalling the PE queue behind each exp
                    for t in range(NT):
                        nc.tensor.matmul(
                            avp[:],
                            at[:, t * 128 : (t + 1) * 128],
                            vbf[:, t * (D + 1) : (t + 1) * (D + 1)],
                            start=(t == 0),
                            stop=(t == NT - 1),
                        )

                    # epilogue: normalize by the ones-column sum, store
                    rden = fpool.tile([128, 1], f32, name="rden", tag="rden")
                    nc.vector.reciprocal(rden[:], avp[:, D : D + 1])
                    osb = fpool.tile([128, D], f32, name="osb", tag="osb")
                    nc.scalar.activation(
                        osb[:], avp[:, 0:D], AF.Copy, scale=rden[:]
                    )
                    nc.scalar.dma_start(out_d.ap()[h], osb[:])
                # release in reverse allocation (stack) order
                for _p in (avps, scps, ktps, fpool, apool, vpool, wpool, iopool, prio):
                    _p.release()

    _split_waits(nc)
    return nc


def _get_program():
    if "nc" not in _cache:
        _cache["nc"] = _build_program()
    return _cache["nc"]


def _make_in_maps(query_q, x_k, x_v, regions, bias_same, bias_diff):
    query_q = np.asarray(query_q, dtype=np.float32)
    x_k = np.asarray(x_k, dtype=np.float32)
    x_v = np.asarray(x_v, dtype=np.float32)
    regions_f = np.asarray(regions).astype(np.float32)
    delta = (
        np.asarray(bias_same, dtype=np.float32) - np.asarray(bias_diff, dtype=np.float32)
    ).astype(np.float16)

    in_maps = []
    for core in range(NCORES):
        b = core // 2
        h0 = (core % 2) * HPC
        in_maps.append(
            {
                "q": np.ascontiguousarray(query_q[b, h0 : h0 + HPC]),
                "k": np.ascontiguousarray(x_k[b, h0 : h0 + HPC]),
                "v": np.ascontiguousarray(x_v[b, h0 : h0 + HPC]),
                "regions_f": np.ascontiguousarray(regions_f[b]),
                "delta8": np.ascontiguousarray(delta[h0 : h0 + HPC]),
            }
        )
    return in_maps


def _gather(res):
    out = np.empty((B, H, P, D), np.float32)
    for core in range(NCORES):
        b = core // 2
        h0 = (core % 2) * HPC
        out[b, h0 : h0 + HPC] = res.results[core]["out"]
    return out


def kernel(
    query_q,
    x_k,
    x_v,
    regions,
    t_mask=None,
    n_mask=None,
    max_n=None,
    bias_same=None,
    bias_diff=None,
    **_unused,
):
    from concourse import bass_utils

    nc = _get_program()
    in_maps = _make_in_maps(query_q, x_k, x_v, regions, bias_same, bias_diff)
    res = bass_utils.run_bass_kernel_spmd(nc, in_maps, core_ids=list(range(NCORES)))
    return _gather(res)
